# revision 41
# baseline (speedup 1.0000x reference)
"""ChebConv-with-spatial-attention Trainium2 kernel (v6).

out[t,b,m,o] = relu( sum_{k,n,f} cheb[k,n,m] * s_a[b,n,m] * X[b,n,f,t] * Theta[k,f,o] )

Shapes: B=16, N=512, F=32, T=24, K=3, O=64.  fp32 in/out (bf16 on the wire).

Strategy (8 NeuronCores, data-parallel over batch, 2 batches per core):
  warmup:         4+1 matmuls on a zeroed tile bridge the PE p-state/HAM
                  ramp while the first input DMAs are in flight.
  stage 0:        A_kb[n,m] = cheb_k[n,m] * s_a_b[n,m]   (DVE b0/k0,k2;
                  GpSimd b0/k1 + all of b1)
  stage 1 (PE):   Y_k[(tj,f), m] = sum_n X[b,n,tg*4+tj,f] * A_kb[n,m]
                  - lhsT = X block [128n, 128(tj,f)], rhs = A [128n, 512m]
                  - one PSUM bank per k (yp0a/yp0b alternating, yp1, yp2),
                    accumulated over the 4 n-tiles; separate tiles keep the
                    three evacuations independent (same-tile readers are
                    chained by the tile dep tracker: ~2.1us/group before)
  evac:           yp_k -> ysb_k bf16 (k0,k2 on DVE; k1 on ACT)
  stage 2 (PE):   block-diagonal Theta: th2[k,h][(tj,f),(tj,o32)] packs all
                  4 t's of the group into ONE full-width matmul; 2 o-halves
                  x 3 k-chain = 6 matmuls into op0/op1 (one PSUM bank each,
                  double-buffered so stage 2 never waits on the relu)
  relu:           op0 -> ACT activation, op1 -> DVE tensor_scalar_max, into
                  one ob tile; one merged [128, 1024] store per group.
  out:            OUT [BL, TG, 128=(tj,o32), 2N=(h,m)]; host unscrambles to
                  [T,B,N,c_out].  The last group runs stage 1 k-major so
                  each k's evacuation overlaps stage 1, and its stores are
                  split across idle rings to shorten the drain tail.

Input DMAs are spread over the sync/gpsimd/scalar rings in need-order so the
first A-tiles land ~2us in; the PE is ~90% busy with zero mid-kernel stalls
in the CoreSim timeline.
"""

import sys

sys.path.insert(0, "/opt/trn_rl_repo")

import numpy as np
import ml_dtypes

import concourse.bacc as bacc
import concourse.tile as tile
from concourse import mybir
from concourse.bass_utils import run_bass_kernel_spmd

B, N, F, T, K, O = 16, 512, 32, 24, 3, 64
NC = 8
BL = B // NC          # batches per core = 2
NT = N // 128         # n tiles = 4
TG = T // 4           # t-groups of 4 = 6
FT = F * T            # 768

MM_MODE = "bf16"


def _build_program(mode):
    io_dt = mybir.dt.bfloat16 if mode == "bf16" else mybir.dt.float32
    nc = bacc.Bacc("TRN2", target_bir_lowering=False, debug=False, num_devices=NC)

    # X pre-transposed on host to [BL, N, T, F] so a [128, 128] slice of the
    # free dim covers 4 consecutive t's of all 32 f's.
    X_d = nc.dram_tensor("X", [BL, N, T * F], io_dt, kind="ExternalInput").ap()
    SA_d = nc.dram_tensor("SA", [BL, N, N], io_dt, kind="ExternalInput").ap()
    CH_d = nc.dram_tensor("CH", [K, N, N], io_dt, kind="ExternalInput").ap()
    # Block-diagonal Theta, host-built: [128 rows=(tj,f), K*2*128 cols],
    # col block (k, h) holds Theta[k, f, 32h+o32] at (32tj+f, 32tj+o32).
    TH_d = nc.dram_tensor("TH", [128, K * 2 * 128], io_dt, kind="ExternalInput").ap()
    # Output bf16: [BL, TG, 128=(tj,o32), 2N=(h,m)]; host unscrambles+upcasts.
    OUT_d = nc.dram_tensor("OUT", [BL, TG, 128, 2 * N], io_dt, kind="ExternalOutput").ap()

    def mm(ap):
        return ap.bitcast(mybir.dt.float32r) if mode == "fp32r" else ap

    SY, GP, DV, SC = None, None, None, None  # set inside ctx

    with tile.TileContext(nc) as tc:
        SY, GP, DV, SC = nc.sync, nc.gpsimd, nc.vector, nc.scalar
        with (
            tc.tile_pool(name="const", bufs=1) as cpool,
            tc.tile_pool(name="ypsum1", bufs=1, space="PSUM") as ypool1,
            tc.tile_pool(name="opsum", bufs=2, space="PSUM") as opool,
            tc.tile_pool(name="ysb", bufs=6) as ysbpool,
            tc.tile_pool(name="osb", bufs=6) as osbpool,
        ):
            xsb = cpool.tile([128, BL * NT * FT], io_dt, tag="xsb")
            chsb = cpool.tile([128, K * NT * N], io_dt, tag="chsb")
            sasb = cpool.tile([128, BL * NT * N], io_dt, tag="sasb")
            asb = cpool.tile([128, K * BL * NT * N], io_dt, tag="asb")
            thsb = cpool.tile([128, K * 2 * 128], io_dt, tag="thsb")
            wsb = cpool.tile([128, N], io_dt, tag="wsb")

            def xoff(b, n4):
                return (b * NT + n4) * FT

            def choff(k, n4):
                return (k * NT + n4) * N

            def saoff(b, n4):
                return (b * NT + n4) * N

            def aoff(k, b, n4):
                return ((k * BL + b) * NT + n4) * N

            # ---- warmup: bridge the PE p-state ramp on zeros while the
            # first input DMAs fly. Emitted first so they head the PE queue.
            DV.memset(wsb[:, :], 0)
            wp = opool.tile([128, N], mybir.dt.float32, tag="op0", name="wp")
            for i in range(4):
                nc.tensor.matmul(
                    wp[:, 0:N], mm(wsb[:, 0:128]), mm(wsb[:, 0:N]),
                    start=True, stop=True,
                )
            nc.tensor.matmul(
                wp[:, 0:256], mm(wsb[:, 0:128]), mm(wsb[:, 0:256]),
                start=True, stop=True,
            )

            # ---- input DMAs, in need-order, spread over rings ----
            def dma_ch(eng, k, n4):
                eng.dma_start(
                    chsb[:, choff(k, n4):choff(k, n4) + N],
                    CH_d[k, n4 * 128:(n4 + 1) * 128, :],
                )

            def dma_sa(eng, b, n4):
                eng.dma_start(
                    sasb[:, saoff(b, n4):saoff(b, n4) + N],
                    SA_d[b, n4 * 128:(n4 + 1) * 128, :],
                )

            def dma_x(eng, b, n4):
                eng.dma_start(
                    xsb[:, xoff(b, n4):xoff(b, n4) + FT],
                    X_d[b, n4 * 128:(n4 + 1) * 128, :],
                )

            # head: need-ordered input DMAs.  SY carries sa/x for b0, GP
            # carries the ch k1/k2 blocks before switching to TT work, SC
            # (busy with the act-table load until ~1.5us) takes the ch k0
            # stragglers.  DVE does the k0/k2 A-mults, GP the k1 ones.
            dma_ch(SY, 0, 0)
            dma_sa(GP, 0, 0)
            dma_x(SY, 0, 0)
            dma_ch(GP, 1, 0)
            dma_ch(SY, 2, 0)
            dma_ch(GP, 1, 1)
            dma_ch(SY, 0, 1)
            dma_ch(GP, 1, 2)
            dma_ch(SY, 2, 1)
            dma_ch(GP, 1, 3)
            dma_sa(SY, 0, 2)
            dma_sa(SC, 0, 1)
            dma_x(SC, 0, 1)
            dma_ch(SY, 0, 2)
            dma_sa(SY, 0, 3)
            dma_ch(SC, 2, 2)
            dma_x(SC, 0, 2)
            dma_ch(SY, 2, 3)
            dma_x(SY, 0, 3)
            dma_ch(SC, 0, 3)

            # ---- stage 0, b0 (need-ordered; DVE k0,k2; GP k1 after its
            # DMA issue work) ----
            def a_mult(eng, k, b, n4):
                eng.tensor_mul(
                    asb[:, aoff(k, b, n4):aoff(k, b, n4) + N],
                    chsb[:, choff(k, n4):choff(k, n4) + N],
                    sasb[:, saoff(b, n4):saoff(b, n4) + N],
                )

            for n4 in range(NT):
                a_mult(DV, 0, 0, n4)
                a_mult(DV, 2, 0, n4)
            for n4 in range(NT):
                a_mult(GP, 1, 0, n4)

            # ---- remaining input DMAs (b1 + theta), then stage 0 b1 ----
            SC.dma_start(thsb[:, :], TH_d)
            dma_x(SY, 1, 0)
            dma_sa(SY, 1, 0)
            dma_x(SC, 1, 1)
            dma_sa(SC, 1, 1)
            dma_x(SY, 1, 2)
            dma_sa(SY, 1, 2)
            dma_x(SC, 1, 3)
            dma_sa(SC, 1, 3)

            for n4 in range(NT):
                for k in range(K):
                    a_mult(GP, k, 1, n4)

            groups = [(b, tg) for b in range(BL) for tg in range(TG)]

            def stage1(b, tg, last=False):
                # one PSUM tile per k so the three evac copies read distinct
                # tiles (same-tile readers get chained by the dep tracker)
                par = 'ab'[(b * TG + tg) % 2]
                yps = [ypool1.tile([128, N], mybir.dt.float32, tag=f"yp0{par}", name="yp0"),
                       ypool1.tile([128, N], mybir.dt.float32, tag="yp1", name="yp1"),
                       ypool1.tile([128, N], mybir.dt.float32, tag="yp2", name="yp2")]

                def mm1(k, n4):
                    xw = xsb[:, xoff(b, n4) + tg * 128: xoff(b, n4) + (tg + 1) * 128]
                    nc.tensor.matmul(
                        yps[k][:, :],
                        mm(xw),
                        mm(asb[:, aoff(k, b, n4):aoff(k, b, n4) + N]),
                        start=(n4 == 0),
                        stop=(n4 == NT - 1),
                    )

                ysbs = [ysbpool.tile([128, N], io_dt, tag=f"ysb{k}", name=f"ysb{k}") for k in range(K)]
                if not last:
                    for n4 in range(NT):
                        for k in range(K):
                            mm1(k, n4)
                    DV.tensor_copy(ysbs[0][:, :], yps[0][:, :])
                    SC.copy(ysbs[1][:, :], yps[1][:, :])
                    DV.tensor_copy(ysbs[2][:, :], yps[2][:, :])
                else:
                    # k-major: each k-chain closes early so its evac overlaps
                    # the rest of stage 1 (shortens the drain tail)
                    for n4 in range(NT):
                        mm1(1, n4)
                    SC.copy(ysbs[1][:, :], yps[1][:, :])
                    for n4 in range(NT):
                        mm1(2, n4)
                    DV.tensor_copy(ysbs[2][:, :], yps[2][:, :])
                    for n4 in range(NT):
                        mm1(0, n4)
                    DV.tensor_copy(ysbs[0][:, :], yps[0][:, :])
                return ysbs

            out_ring = [SY, GP]

            def stage2(b, tg, ysbs, last=False):
                # Block-diag Theta: one matmul covers all 4 t's for (k, h);
                # chain over k per o-half h, each half in its own PSUM bank
                # (separate tiles so the two relus read distinct tiles).
                ops = [opool.tile([128, N], mybir.dt.float32, tag=f"op{h}", name=f"op{h}")
                       for h in range(2)]
                g = b * TG + tg
                # last group: consume k's in evac-completion order (k1 and k2
                # were evacuated during stage 1; k0 lands just in time),
                # h-major so op0 closes 3 MMs early and ACT can run both
                # relus back-to-back
                korder = [1, 2, 0] if last else [0, 1, 2]
                kh = [(k, h) for h in range(2) for k in korder] if last else \
                     [(k, h) for k in korder for h in range(2)]
                for k, h in kh:
                    nc.tensor.matmul(
                        ops[h][:, :],
                        mm(thsb[:, (k * 2 + h) * 128:(k * 2 + h) * 128 + 128]),
                        mm(ysbs[k][:, :]),
                        start=(k == korder[0]),
                        stop=(k == korder[-1]),
                    )
                if not last:
                    # h0 on ACT, h1 on DVE: the op tiles (bufs=1) are released
                    # in parallel so the next group's stage-2 never waits
                    ob = osbpool.tile([128, 2 * N], io_dt, tag="ob")
                    SC.activation(ob[:, 0:N], ops[0][:, :], mybir.ActivationFunctionType.Relu)
                    DV.tensor_scalar_max(ob[:, N:2 * N], ops[1][:, :], 0.0)
                    out_ring[g % 2].dma_start(OUT_d[b, tg], ob[:, :])
                else:
                    # split the final relu+store across engines/rings to
                    # shorten the drain tail (separate ob tiles: same-tile
                    # writers would serialize)
                    ob0 = osbpool.tile([128, N], io_dt, tag="ob0", name="ob0")
                    ob1 = osbpool.tile([128, N], io_dt, tag="ob1", name="ob1")
                    DV.tensor_scalar_max(ob0[:, :], ops[0][:, :], 0.0)
                    GP.dma_start(OUT_d[b, tg, :, 0:N], ob0[:, :])
                    SC.activation(ob1[:, :], ops[1][:, :], mybir.ActivationFunctionType.Relu)
                    SC.dma_start(OUT_d[b, tg, :, N:N + N // 2], ob1[:, 0:N // 2])
                    SY.dma_start(OUT_d[b, tg, :, N + N // 2:2 * N], ob1[:, N // 2:N])

            # software-pipeline: stage2(g-1) is emitted after stage1(g) so the
            # PE never waits on the PSUM->SBUF evacuation of the current group
            prev = None
            for g, (b, tg) in enumerate(groups):
                ysb = stage1(b, tg, last=(g == len(groups) - 1))
                if prev is not None:
                    stage2(*prev)
                prev = (b, tg, ysb)
            stage2(*prev, last=True)

    nc.compile()
    return nc


_prog_cache = {}


def _get_program(mode):
    if mode not in _prog_cache:
        _prog_cache[mode] = _build_program(mode)
    return _prog_cache[mode]


def _prep_inputs(X, s_a, cheb, Theta):
    np_dt = ml_dtypes.bfloat16 if MM_MODE == "bf16" else np.float32
    Xh = np.ascontiguousarray(X.transpose(0, 1, 3, 2)).reshape(B, N, T * F).astype(np_dt)
    sah = np.ascontiguousarray(s_a).astype(np_dt)
    chh = np.ascontiguousarray(cheb).astype(np_dt)
    # th2[32*tj+f, (k,h)*128 + 32*tj+o32] = Theta[k, f, 32*h+o32]
    th2 = np.zeros((128, K * 2 * 128), dtype=np.float32)
    for k in range(K):
        for h in range(2):
            blk = th2[:, (k * 2 + h) * 128:(k * 2 + h) * 128 + 128]
            for tj in range(4):
                blk[32 * tj:32 * tj + 32, 32 * tj:32 * tj + 32] = Theta[k, :, 32 * h:32 * h + 32]
    th2 = th2.astype(np_dt)
    in_maps = []
    for c in range(NC):
        lo, hi = c * BL, (c + 1) * BL
        in_maps.append({"X": Xh[lo:hi], "SA": sah[lo:hi], "CH": chh, "TH": th2})
    return in_maps


def kernel(X, s_a, cheb, Theta):
    in_maps = _prep_inputs(X, s_a, cheb, Theta)
    nc = _get_program(MM_MODE)
    res = run_bass_kernel_spmd(nc, in_maps, list(range(NC)))
    # per-core OUT: [BL, TG, 128, 2N] bf16 (partition = 32*tj + o32,
    # free = (h, m))
    out = np.concatenate([r["OUT"] for r in res.results], axis=0).astype(np.float32)
    out = out.reshape(B, TG, 4, 32, 2, N)
    # [b, tg, tj, o32, h, m] -> [t=(tg,tj), b, n=m, o=(h,o32)]
    out = out.transpose(1, 2, 0, 5, 4, 3).reshape(T, B, N, O)
    return np.ascontiguousarray(out)
